# revision 2
# baseline (speedup 1.0000x reference)
"""BSplineBasis kernel v6: closed-form truncated-power evaluation, no masks.

For the uniform cubic B-spline with x in [0,1) (cells j in {4,5,6}),
with u = 2.5x - 0.75 (cell-5 local coordinate), every output slot is an
exact small combination of (relu-)cubes -- the truncated-power form:
  L = relu(-u)   R = relu(u-1)   M = relu(1-u)   Ms = relu(u)
  N = 2-u        N2 = 1+u        (plain, always positive)
  og0 = 0                        og7 = 0
  og1 = L^3/6                    og6 = R^3/6
  og2 = (M^3 - 4L^3)/6           og5 = (Ms^3 - 4R^3)/6
  og3 = (6L^3 - 4M^3 + N^3)/6    og4 = (6R^3 - 4Ms^3 + N2^3)/6
No data-dependent control flow at all: Ln(0) -> -inf -> Exp -> 0 makes the
relu terms vanish outside their support automatically.
Cubes via ACT LUT: X^3*s = Exp(3*Ln(X) + ln s).  t-side and mirrored w-side
quantities are packed into [P, 2W] tiles so one instruction covers both; the
interleaved out slots are written through strided pair views (1 write/slot).
Slots 0,7 pre-zeroed once per out buffer (never written in the loop).
Sharding: pure data-parallel over batch across 8 cores (4096 rows each).
"""

import sys

sys.path.insert(0, "/opt/trn_rl_repo")

import math
import numpy as np

import concourse.bacc as bacc
import concourse.tile as tile
from concourse import mybir
from concourse.bass_types import AP

N_CORES = 8
P = 128
F = 512
E = 8
A = 2
W = A * F

AF = mybir.ActivationFunctionType
OP = mybir.AluOpType

LN6 = math.log(6.0)
LN46 = math.log(4.0 / 6.0)


def _build_program(rows: int, consts: tuple, repeat: int = 1):
    inv_h, u_bias, g5, g6 = consts
    u_scale = float(inv_h)
    u_off = float(u_bias)
    nc = bacc.Bacc("TRN2", target_bir_lowering=False, debug=False,
                   num_devices=N_CORES)
    f32 = mybir.dt.float32
    x = nc.declare_dram_parameter("x", [rows, F], f32, isOutput=False)
    out = nc.declare_dram_parameter("out", [rows, F * E], f32, isOutput=True)
    rows_per_tile = A * P
    ntiles = rows // rows_per_tile
    OUT_BUFS = 3

    xv = x.rearrange("(n a p) f -> n p a f", a=A, p=P)
    ov = out.rearrange("(n a p) g -> n p a g", a=A, p=P)

    # LUT activations (Exp/Ln/Relu) need float biases as const APs.
    for val in (-LN6, LN46):
        ctens = nc.alloc_sbuf_tensor(f"const-f32-{val}", [128, 1], f32)
        nc.gpsimd.memset(ctens.ap(), val)
        nc.const_aps.aps[(f32, val)] = ctens.ap()
    nc.all_engine_barrier()

    def pair_view(ot, offset, stride):
        """[P, 2, W] view of out tile: pair element k at free addr
        8q + offset + k*stride."""
        base = ot[:, :]
        return AP(base.tensor, base.offset + offset,
                  [base.ap[0], [stride, 2], [8, W]])

    with tile.TileContext(nc) as tc:
        with (
            tc.tile_pool(name="io", bufs=3) as io,
            tc.tile_pool(name="mid", bufs=2) as mid,
            tc.tile_pool(name="outp", bufs=1) as outp,
        ):
            out_tiles = []
            for bi in range(OUT_BUFS):
                ot0 = outp.tile([P, W * E], f32, tag=f"out{bi}")
                og0 = ot0.rearrange("p (q e) -> p q e", e=E)
                nc.vector.memset(og0[:, :, 0], 0.0)
                nc.vector.memset(og0[:, :, 7], 0.0)
                out_tiles.append(ot0)

            for it, i in enumerate(
                    [i for _ in range(repeat) for i in range(ntiles)]):
                xt = io.tile([P, W], f32, tag="x")
                nc.sync.dma_start(out=xt.rearrange("p (a f) -> p a f", a=A), in_=xv[i])

                ot = out_tiles[it % OUT_BUFS]

                u4p = mid.tile([P, W], f32, tag="u4p")
                nc.scalar.activation(u4p, xt, AF.Copy, bias=u_off, scale=u_scale)

                LR = mid.tile([P, 2 * W], f32, tag="LR")
                nc.vector.tensor_scalar(LR[:, :W], u4p, -1.0, 0.0, OP.mult, OP.max)
                nc.vector.tensor_scalar(LR[:, W:], u4p, 1.0, 0.0,
                                        OP.subtract, OP.max)
                MM = mid.tile([P, 2 * W], f32, tag="MM")
                nc.scalar.activation(MM[:, :W], u4p, AF.Relu, bias=1.0, scale=-1.0)
                nc.vector.tensor_scalar(MM[:, W:], u4p, 0.0, None, OP.max)
                NN = mid.tile([P, 2 * W], f32, tag="NN")
                nc.vector.tensor_scalar(NN[:, :W], u4p, -1.0, 2.0, OP.mult, OP.add)
                nc.vector.tensor_scalar(NN[:, W:], u4p, 1.0, None, OP.add)

                # in-place Ln then in-place Exp: X -> ln X -> X^3/6
                nc.scalar.activation(LR, LR, AF.Ln)
                nc.scalar.activation(MM, MM, AF.Ln)
                nc.scalar.activation(NN, NN, AF.Ln)
                nc.scalar.activation(LR, LR, AF.Exp, bias=-LN6, scale=3.0)
                # LR now holds L^3/6 | R^3/6
                nc.vector.tensor_copy(pair_view(ot, 1, 5), LR)   # og1, og6
                LR34 = mid.tile([P, 2 * W], f32, tag="LR34")
                nc.vector.tensor_scalar(LR34, LR, 4.0, None, OP.mult)
                LR3s = mid.tile([P, 2 * W], f32, tag="LR3s")
                nc.vector.tensor_scalar(LR3s, LR, 6.0, None, OP.mult)
                nc.scalar.activation(MM, MM, AF.Exp, bias=-LN6, scale=3.0)
                nc.scalar.activation(NN, NN, AF.Exp, bias=-LN6, scale=3.0)

                # og2 = M^3/6 - 4L^3/6 ; og5 = Ms^3/6 - 4R^3/6
                nc.vector.tensor_tensor(pair_view(ot, 2, 3), MM, LR34,
                                        OP.subtract)
                # og3 = L^3 - 4M^3/6 + N^3/6 ; og4 mirrored
                MM34 = MM
                nc.vector.tensor_scalar(MM34, MM, 4.0, None, OP.mult)
                nc.vector.tensor_tensor(LR3s, LR3s, MM34, OP.subtract)
                nc.vector.tensor_tensor(pair_view(ot, 3, 1), LR3s, NN, OP.add)

                nc.sync.dma_start(
                    out=ov[i], in_=ot.rearrange("p (a g) -> p a g", a=A))

    nc.compile()
    return nc


_PROGRAM_CACHE: dict = {}


def _get_program(rows: int, consts: tuple):
    key = (rows, consts)
    if key not in _PROGRAM_CACHE:
        _PROGRAM_CACHE[key] = _build_program(rows, consts)
    return _PROGRAM_CACHE[key]


def kernel(x, grid):
    from concourse.bass_utils import run_bass_kernel_spmd

    x = np.ascontiguousarray(np.asarray(x, dtype=np.float32))
    grid = np.asarray(grid, dtype=np.float32)
    n, f = x.shape
    assert f == F and n % (N_CORES * A * P) == 0, (n, f)
    rows = n // N_CORES

    g4 = np.float32(grid[0, 4])
    g5 = np.float32(grid[0, 5])
    g6 = np.float32(grid[0, 6])
    h = np.float32(grid[0, 5] - grid[0, 4])
    inv_h = np.float32(np.float32(1.0) / h)
    u_bias = np.float32(-np.float64(g4) * np.float64(inv_h) - 1.0)

    consts = (float(inv_h), float(u_bias), float(g5), float(g6))
    nc = _get_program(rows, consts)
    in_maps = [{"x": x[c * rows:(c + 1) * rows]} for c in range(N_CORES)]
    res = run_bass_kernel_spmd(nc, in_maps, list(range(N_CORES)))
    return np.concatenate([res.results[c]["out"] for c in range(N_CORES)], axis=0)


# revision 3
# speedup vs baseline: 2.0982x; 2.0982x over previous
"""BSplineBasis kernel v6: closed-form truncated-power evaluation, no masks.

For the uniform cubic B-spline with x in [0,1) (cells j in {4,5,6}),
with u = 2.5x - 0.75 (cell-5 local coordinate), every output slot is an
exact small combination of (relu-)cubes -- the truncated-power form:
  L = relu(-u)   R = relu(u-1)   M = relu(1-u)   Ms = relu(u)
  N = 2-u        N2 = 1+u        (plain, always positive)
  og0 = 0                        og7 = 0
  og1 = L^3/6                    og6 = R^3/6
  og2 = (M^3 - 4L^3)/6           og5 = (Ms^3 - 4R^3)/6
  og3 = (6L^3 - 4M^3 + N^3)/6    og4 = (6R^3 - 4Ms^3 + N2^3)/6
No data-dependent control flow at all: Ln(0) -> -inf -> Exp -> 0 makes the
relu terms vanish outside their support automatically.
Cubes via ACT LUT: X^3*s = Exp(3*Ln(X) + ln s).  t-side and mirrored w-side
quantities are packed into [P, 2W] tiles so one instruction covers both; the
interleaved out slots are written through strided pair views (1 write/slot).
Slots 0,7 pre-zeroed once per out buffer (never written in the loop).
Sharding: pure data-parallel over batch across 8 cores (4096 rows each).
"""

import sys

sys.path.insert(0, "/opt/trn_rl_repo")

import math
import numpy as np

import concourse.bacc as bacc
import concourse.tile as tile
from concourse import mybir
from concourse.bass_types import AP

N_CORES = 8
P = 128
F = 512
E = 8
A = 2
W = A * F

AF = mybir.ActivationFunctionType
OP = mybir.AluOpType

LN6 = math.log(6.0)
LN46 = math.log(4.0 / 6.0)


def _build_program(rows: int, consts: tuple, repeat: int = 1):
    inv_h, u_bias, g5, g6 = consts
    u_scale = float(inv_h)
    u_off = float(u_bias)
    nc = bacc.Bacc("TRN2", target_bir_lowering=False, debug=False,
                   num_devices=N_CORES)
    f32 = mybir.dt.float32
    x = nc.declare_dram_parameter("x", [rows, F], f32, isOutput=False)
    out = nc.declare_dram_parameter("out", [rows, F * E], f32, isOutput=True)
    rows_per_tile = A * P
    ntiles = rows // rows_per_tile
    OUT_BUFS = 3

    xv = x.rearrange("(n a p) f -> n p a f", a=A, p=P)
    ov = out.rearrange("(n a p) g -> n p a g", a=A, p=P)

    # LUT activations (Exp/Ln/Relu) need float biases as const APs.
    for val in (-LN6, LN46):
        ctens = nc.alloc_sbuf_tensor(f"const-f32-{val}", [128, 1], f32)
        nc.gpsimd.memset(ctens.ap(), val)
        nc.const_aps.aps[(f32, val)] = ctens.ap()
    nc.all_engine_barrier()

    def pair_view(ot, offset, stride):
        """[P, 2, W] view of out tile: pair element k at free addr
        8q + offset + k*stride."""
        base = ot[:, :]
        return AP(base.tensor, base.offset + offset,
                  [base.ap[0], [stride, 2], [8, W]])

    with tile.TileContext(nc) as tc:
        with (
            tc.tile_pool(name="io", bufs=3) as io,
            tc.tile_pool(name="mid", bufs=2) as mid,
            tc.tile_pool(name="outp", bufs=1) as outp,
        ):
            out_tiles = []
            for bi in range(OUT_BUFS):
                ot0 = outp.tile([P, W * E], f32, tag=f"out{bi}")
                og0 = ot0.rearrange("p (q e) -> p q e", e=E)
                nc.vector.memset(og0[:, :, 0], 0.0)
                nc.vector.memset(og0[:, :, 7], 0.0)
                out_tiles.append(ot0)

            for it, i in enumerate(
                    [i for _ in range(repeat) for i in range(ntiles)]):
                xt = io.tile([P, W], f32, tag="x")
                nc.sync.dma_start(out=xt.rearrange("p (a f) -> p a f", a=A), in_=xv[i])

                ot = out_tiles[it % OUT_BUFS]

                u4p = mid.tile([P, W], f32, tag="u4p")
                nc.scalar.activation(u4p, xt, AF.Copy, bias=u_off, scale=u_scale)

                LR = mid.tile([P, 2 * W], f32, tag="LR")
                nc.vector.tensor_scalar(LR[:, :W], u4p, -1.0, 0.0, OP.mult, OP.max)
                nc.vector.tensor_scalar(LR[:, W:], u4p, 1.0, 0.0,
                                        OP.subtract, OP.max)
                MN = mid.tile([P, 4 * W], f32, tag="MN")
                MM = MN[:, :2 * W]
                NN = MN[:, 2 * W:]
                nc.scalar.activation(MM[:, :W], u4p, AF.Relu, bias=1.0, scale=-1.0)
                nc.vector.tensor_scalar(MM[:, W:], u4p, 0.0, None, OP.max)
                nc.vector.tensor_scalar(NN[:, :W], u4p, -1.0, 2.0, OP.mult, OP.add)
                nc.vector.tensor_scalar(NN[:, W:], u4p, 1.0, None, OP.add)

                # in-place Ln then in-place Exp: X -> ln X -> X^3/6
                nc.scalar.activation(LR, LR, AF.Ln)
                nc.scalar.activation(MN, MN, AF.Ln)
                nc.scalar.activation(LR, LR, AF.Exp, bias=-LN6, scale=3.0)
                # LR now holds L^3/6 | R^3/6
                nc.vector.tensor_copy(pair_view(ot, 1, 5), LR)   # og1, og6
                LR34 = mid.tile([P, 2 * W], f32, tag="LR34")
                nc.vector.tensor_scalar(LR34, LR, 4.0, None, OP.mult)
                LR3s = mid.tile([P, 2 * W], f32, tag="LR3s")
                nc.vector.tensor_scalar(LR3s, LR, 6.0, None, OP.mult)
                nc.scalar.activation(MN, MN, AF.Exp, bias=-LN6, scale=3.0)

                # og2 = M^3/6 - 4L^3/6 ; og5 = Ms^3/6 - 4R^3/6
                nc.vector.tensor_tensor(pair_view(ot, 2, 3), MM, LR34,
                                        OP.subtract)
                # og3 = L^3 - 4M^3/6 + N^3/6 ; og4 mirrored
                MM34 = MM
                nc.vector.tensor_scalar(MM34, MM, 4.0, None, OP.mult)
                nc.vector.tensor_tensor(LR3s, LR3s, MM34, OP.subtract)
                nc.vector.tensor_tensor(pair_view(ot, 3, 1), LR3s, NN, OP.add)

                otv = ot.rearrange("p (a g) -> p a g", a=A)
                nc.sync.dma_start(out=ov[i][:, 0:1], in_=otv[:, 0:1])
                nc.gpsimd.dma_start(out=ov[i][:, 1:2], in_=otv[:, 1:2])

    nc.compile()
    return nc


_PROGRAM_CACHE: dict = {}


def _get_program(rows: int, consts: tuple):
    key = (rows, consts)
    if key not in _PROGRAM_CACHE:
        _PROGRAM_CACHE[key] = _build_program(rows, consts)
    return _PROGRAM_CACHE[key]


def kernel(x, grid):
    from concourse.bass_utils import run_bass_kernel_spmd

    x = np.ascontiguousarray(np.asarray(x, dtype=np.float32))
    grid = np.asarray(grid, dtype=np.float32)
    n, f = x.shape
    assert f == F and n % (N_CORES * A * P) == 0, (n, f)
    rows = n // N_CORES

    g4 = np.float32(grid[0, 4])
    g5 = np.float32(grid[0, 5])
    g6 = np.float32(grid[0, 6])
    h = np.float32(grid[0, 5] - grid[0, 4])
    inv_h = np.float32(np.float32(1.0) / h)
    u_bias = np.float32(-np.float64(g4) * np.float64(inv_h) - 1.0)

    consts = (float(inv_h), float(u_bias), float(g5), float(g6))
    nc = _get_program(rows, consts)
    in_maps = [{"x": x[c * rows:(c + 1) * rows]} for c in range(N_CORES)]
    res = run_bass_kernel_spmd(nc, in_maps, list(range(N_CORES)))
    return np.concatenate([res.results[c]["out"] for c in range(N_CORES)], axis=0)
